# revision 9
# baseline (speedup 1.0000x reference)
"""GAT-style 2-layer GNN message passing on 8 Trainium2 NeuronCores.

Math note: for this reference, the segment-softmax ratio
  num/den = (sum_j h[j]*exp((s_l[i]+s_r[j])/2d)) / (sum_j exp((s_l[i]+s_r[j])/2d))
has the destination factor exp(s_l[i]/2d) cancel, so per layer we only need
  a[i] = (sum_{j in N(i)} w_j*h_j) / (sum_{j in N(i)} w_j),  w_j = exp(s_r[j]/2d).

Sharding: nodes split into 8 contiguous destination ranges (6250/core).
Each core builds table rows [g=w*h (256) | w (4) | pad] (bf16, 768B) for its
own nodes, an AllGather shares the full 50k-row table, then each core
aggregates its own destinations: per-edge dma_gather of source rows + one-hot
matmul (segment-sum into PSUM), then divide / layernorm / leaky-relu.
Edges are pre-sorted by destination on the host; indices are int16 so the
table is gathered via two base pointers (src < 32768 and src >= 32768).
"""

import os
import sys

import numpy as np
import ml_dtypes

sys.path.insert(0, "/opt/trn_rl_repo")

import concourse.bacc as bacc
import concourse.bass as bass
import concourse.mybir as mybir
import concourse.tile as tile
from concourse.bass_utils import run_bass_kernel_spmd

BF16 = mybir.dt.bfloat16
F32 = mybir.dt.float32
I16 = mybir.dt.int16

N, DIN, E = 50000, 128, 800000
H, D = 4, 64
F = H * D  # 256
NCORE = 8
NPC = N // NCORE  # 6250
NBLK = (NPC + 127) // 128  # 49 destination blocks per core
EPS = 1e-5
SLOPE = 0.01
ROWE = 384  # table row: 256 g + 4 w + 124 pad (bf16) = 768 bytes
SPLIT = 32768  # int16 index limit

LAST_RESULTS = None

AF = mybir.ActivationFunctionType
ALU = mybir.AluOpType


def _host_prep(x, edge_index, W0, b0, W1, b1, att0, att1, gamma, beta):
    """Build all per-core and shared input arrays."""
    bf16 = ml_dtypes.bfloat16
    dst = np.asarray(edge_index[0], dtype=np.int64)
    src = np.asarray(edge_index[1], dtype=np.int64)
    x = np.asarray(x, dtype=np.float32)

    # --- per-core edge grids ---
    per_core = []
    nb1_max, nb2_max = 0, 0
    for c in range(NCORE):
        lo_node, hi_node = c * NPC, (c + 1) * NPC
        m = (dst >= lo_node) & (dst < hi_node)
        ld = (dst[m] - lo_node).astype(np.int64)
        s = src[m]
        blocks = []
        for b in range(NBLK):
            bm = (ld >> 7) == b
            sb = s[bm]
            lb = (ld[bm] & 127).astype(np.int64)
            lo_m = sb < SPLIT
            runs = (
                (sb[lo_m], lb[lo_m]),
                (sb[~lo_m] - SPLIT, lb[~lo_m]),
            )
            blocks.append(runs)
            nb1_max = max(nb1_max, -(-len(runs[0][0]) // 128))
            nb2_max = max(nb2_max, -(-len(runs[1][0]) // 128))
        per_core.append(blocks)
    NB1, NB2 = max(nb1_max, 1), max(nb2_max, 1)
    NBT = NB1 + NB2

    eidx_all, dloc_all = [], []
    for c in range(NCORE):
        eidx = np.zeros((128, NBLK * NBT * 8), dtype=np.int16)
        dloc = np.full((128, NBLK * NBT), 255.0, dtype=np.float32)
        for b in range(NBLK):
            for r, nb in ((0, NB1), (1, NB2)):
                sidx, lb = per_core[c][b][r]
                n = len(sidx)
                pad = nb * 128 - n
                si = np.concatenate([sidx, np.zeros(pad, np.int64)]).astype(np.int16)
                dl = np.concatenate([lb, np.full(pad, 255, np.int64)]).astype(
                    np.float32
                )
                c0 = (b * NBT + (0 if r == 0 else NB1)) * 8
                eidx[:, c0 : c0 + nb * 8] = np.tile(
                    si.reshape(nb * 8, 16).T, (8, 1)
                )
                cb = b * NBT + (0 if r == 0 else NB1)
                dloc[:, cb : cb + nb] = dl.reshape(nb, 128).T
        eidx_all.append(eidx)
        dloc_all.append(dloc)

    # --- xT slices (bf16, padded to 49*128 columns) ---
    xts = []
    for c in range(NCORE):
        xt = np.zeros((DIN, NBLK * 128), dtype=bf16)
        xt[:, :NPC] = x[c * NPC : (c + 1) * NPC].T.astype(bf16)
        xts.append(xt)

    shared = {
        "w0t": np.ascontiguousarray(np.asarray(W0, np.float32).T).astype(bf16),
        "w1t": np.ascontiguousarray(np.asarray(W1, np.float32).T).astype(bf16),
        "b0b": np.tile(np.asarray(b0, np.float32)[None, :], (128, 1)),
        "b1b": np.tile(np.asarray(b1, np.float32)[None, :], (128, 1)),
        "ar0": np.tile(
            np.asarray(att0, np.float32)[0, :, D:].reshape(-1)[None, :], (128, 1)
        ),
        "ar1": np.tile(
            np.asarray(att1, np.float32)[0, :, D:].reshape(-1)[None, :], (128, 1)
        ),
        "gmb": np.tile(np.asarray(gamma, np.float32)[None, :], (128, 1)),
        "btb": np.tile(np.asarray(beta, np.float32)[None, :], (128, 1)),
        "iot": np.tile(np.arange(128, dtype=np.float32)[None, :], (128, 1)).astype(
            bf16
        ),
        "idn": np.eye(128, dtype=np.float32).astype(bf16),
    }
    in_maps = []
    for c in range(NCORE):
        m = dict(shared)
        m["xt"] = xts[c]
        m["eidx"] = eidx_all[c]
        m["dloc"] = dloc_all[c]
        in_maps.append(m)
    return in_maps, NB1, NB2


def _build_program(NB1, NB2):
    NBT = NB1 + NB2
    nc = bacc.Bacc("TRN2", target_bir_lowering=False, debug=False, num_devices=NCORE)

    # I/O
    xt_d = nc.dram_tensor("xt", [DIN, NBLK * 128], BF16, kind="ExternalInput")
    eidx_d = nc.dram_tensor("eidx", [128, NBLK * NBT * 8], I16, kind="ExternalInput")
    dloc_d = nc.dram_tensor("dloc", [128, NBLK * NBT], F32, kind="ExternalInput")
    w0t_d = nc.dram_tensor("w0t", [DIN, F], BF16, kind="ExternalInput")
    w1t_d = nc.dram_tensor("w1t", [F, F], BF16, kind="ExternalInput")
    b0b_d = nc.dram_tensor("b0b", [128, F], F32, kind="ExternalInput")
    b1b_d = nc.dram_tensor("b1b", [128, F], F32, kind="ExternalInput")
    ar0_d = nc.dram_tensor("ar0", [128, F], F32, kind="ExternalInput")
    ar1_d = nc.dram_tensor("ar1", [128, F], F32, kind="ExternalInput")
    gmb_d = nc.dram_tensor("gmb", [128, F], F32, kind="ExternalInput")
    btb_d = nc.dram_tensor("btb", [128, F], F32, kind="ExternalInput")
    iot_d = nc.dram_tensor("iot", [128, 128], BF16, kind="ExternalInput")
    idn_d = nc.dram_tensor("idn", [128, 128], BF16, kind="ExternalInput")
    out_d = nc.dram_tensor("out", [NPC, D], F32, kind="ExternalOutput")

    # Internal DRAM
    tbl_own = [nc.dram_tensor(f"tbl_own{l}", [NPC, ROWE], BF16) for l in range(2)]
    tbl_full = [
        nc.dram_tensor(f"tbl_full{l}", [N, ROWE], BF16, addr_space="Shared")
        for l in range(2)
    ]

    groups = [list(range(NCORE))]

    with tile.TileContext(nc) as tc:
        with (
            tc.tile_pool(name="const", bufs=1) as cpool,
            tc.tile_pool(name="stat", bufs=3) as spool,
            tc.tile_pool(name="hbuf", bufs=3) as hpool,
            tc.tile_pool(name="small", bufs=4) as smpool,
            tc.tile_pool(name="tblt", bufs=3) as tbpool,
            tc.tile_pool(name="glo", bufs=2) as glopool,
            tc.tile_pool(name="ghi", bufs=2) as ghipool,
            tc.tile_pool(name="oh", bufs=4) as ohpool,
            tc.tile_pool(name="post", bufs=3) as postpool,
            tc.tile_pool(name="gemm", bufs=2, space="PSUM") as gpsum,
            tc.tile_pool(name="agg", bufs=2, space="PSUM") as apsum,
            tc.tile_pool(name="tp", bufs=2, space="PSUM") as tpsum,
        ):
            # ---- load constants ----
            def cload(dram, shape, dtype):
                t = cpool.tile(shape, dtype, tag=dram.name)
                nc.sync.dma_start(out=t[:], in_=dram[:, :])
                return t

            xt_s = cload(xt_d, [DIN, NBLK * 128], BF16)
            eidx_s = cload(eidx_d, [128, NBLK * NBT * 8], I16)
            dloc_s = cload(dloc_d, [128, NBLK * NBT], F32)
            w0t_s = cload(w0t_d, [DIN, F], BF16)
            w1t_s = []
            for cch in range(2):
                t = cpool.tile([128, F], BF16, tag=f"w1t{cch}")
                nc.sync.dma_start(
                    out=t[:], in_=w1t_d[cch * 128 : (cch + 1) * 128, :]
                )
                w1t_s.append(t)
            b0b_s = cload(b0b_d, [128, F], F32)
            b1b_s = cload(b1b_d, [128, F], F32)
            ar0_s = cload(ar0_d, [128, F], F32)
            ar1_s = cload(ar1_d, [128, F], F32)
            gmb_s = cload(gmb_d, [128, F], F32)
            btb_s = cload(btb_d, [128, F], F32)
            iot_s = cload(iot_d, [128, 128], BF16)
            idn_s = cload(idn_d, [128, 128], BF16)
            x1_s = cpool.tile([128, NBLK * F], BF16, tag="x1")

            def build_layer(l):
                b_s = b0b_s if l == 0 else b1b_s
                ar_s = ar0_s if l == 0 else ar1_s
                for t in range(NBLK):
                    rows = min(128, NPC - t * 128)
                    ps = gpsum.tile([128, F], F32, tag="gemm")
                    if l == 0:
                        nc.tensor.matmul(
                            ps[:],
                            lhsT=xt_s[:, t * 128 : (t + 1) * 128],
                            rhs=w0t_s[:],
                            start=True,
                            stop=True,
                        )
                    else:
                        for cch in range(2):
                            pt = tpsum.tile([128, 128], BF16, tag="tp")
                            nc.tensor.transpose(
                                pt[:],
                                x1_s[:, t * F + cch * 128 : t * F + (cch + 1) * 128],
                                idn_s[:],
                            )
                            st = spool.tile([128, 128], BF16, tag="stat")
                            nc.vector.tensor_copy(st[:], pt[:])
                            nc.tensor.matmul(
                                ps[:],
                                lhsT=st[:],
                                rhs=w1t_s[cch][:],
                                start=(cch == 0),
                                stop=(cch == 1),
                            )
                    h = hpool.tile([128, F], F32, tag="h")
                    nc.vector.tensor_tensor(h[:], ps[:], b_s[:], op=ALU.add)
                    u = hpool.tile([128, F], F32, tag="u")
                    nc.vector.tensor_tensor(u[:], h[:], ar_s[:], op=ALU.mult)
                    sr = smpool.tile([128, H], F32, tag="sr")
                    scr = hpool.tile([128, F], F32, tag="scr")
                    for hd in range(H):
                        nc.scalar.activation(
                            scr[:, hd * D : (hd + 1) * D],
                            u[:, hd * D : (hd + 1) * D],
                            AF.Lrelu,
                            alpha=SLOPE,
                            accum_out=sr[:, hd : hd + 1],
                        )
                    wv = smpool.tile([128, H], F32, tag="wv")
                    nc.scalar.activation(wv[:], sr[:], AF.Exp, scale=1.0 / (2 * D))
                    tb = tbpool.tile([128, F + H], BF16, tag="tb")
                    for hd in range(H):
                        nc.vector.tensor_scalar_mul(
                            tb[:, hd * D : (hd + 1) * D],
                            h[:, hd * D : (hd + 1) * D],
                            wv[:, hd : hd + 1],
                        )
                    nc.vector.tensor_copy(tb[:, F : F + H], wv[:])
                    nc.sync.dma_start(
                        out=tbl_own[l][t * 128 : t * 128 + rows, 0 : F + H],
                        in_=tb[:rows, :],
                    )
                nc.gpsimd.collective_compute(
                    "AllGather",
                    ALU.bypass,
                    replica_groups=groups,
                    ins=[tbl_own[l][:, :]],
                    outs=[tbl_full[l][:, :]],
                )

            def agg_layer(l):
                for t in range(NBLK):
                    rows = min(128, NPC - t * 128)
                    ps = apsum.tile([128, F + H], F32, tag="agg")
                    nmm = 0
                    for r, nb, pool in ((0, NB1, glopool), (1, NB2, ghipool)):
                        gt = pool.tile([128, nb, ROWE], BF16, tag=f"g{r}")
                        base = (
                            tbl_full[l][0:SPLIT, :]
                            if r == 0
                            else tbl_full[l][SPLIT:N, :]
                        )
                        c0 = (t * NBT + (0 if r == 0 else NB1)) * 8
                        nc.gpsimd.dma_gather(
                            gt[:],
                            base,
                            eidx_s[:, c0 : c0 + nb * 8],
                            nb * 128,
                            nb * 128,
                            ROWE,
                            single_packet=(nb * 128 <= 1024),
                        )
                        for b in range(nb):
                            cb = t * NBT + (0 if r == 0 else NB1) + b
                            oh = ohpool.tile([128, 128], BF16, tag="oh")
                            nc.vector.tensor_scalar(
                                oh[:],
                                iot_s[:],
                                dloc_s[:, cb : cb + 1],
                                None,
                                op0=ALU.is_equal,
                            )
                            nc.tensor.matmul(
                                ps[:],
                                lhsT=oh[:],
                                rhs=gt[:, b, 0 : F + H],
                                start=(nmm == 0),
                                stop=(nmm == NBT - 1),
                            )
                            nmm += 1
                    rec = smpool.tile([128, H], F32, tag="rec")
                    nc.vector.reciprocal(rec[:], ps[:, F : F + H])
                    if l == 0:
                        a0 = postpool.tile([128, F], F32, tag="a0")
                        for hd in range(H):
                            nc.vector.tensor_scalar_mul(
                                a0[:, hd * D : (hd + 1) * D],
                                ps[:, hd * D : (hd + 1) * D],
                                rec[:, hd : hd + 1],
                            )
                        mu = smpool.tile([128, 1], F32, tag="mu")
                        nc.vector.tensor_reduce(
                            mu[:], a0[:], axis=mybir.AxisListType.X, op=ALU.add
                        )
                        nc.vector.tensor_scalar_mul(mu[:], mu[:], 1.0 / F)
                        dd = postpool.tile([128, F], F32, tag="dd")
                        nc.vector.tensor_scalar_sub(dd[:], a0[:], mu[:])
                        vs = smpool.tile([128, 1], F32, tag="vs")
                        scr2 = postpool.tile([128, F], F32, tag="scr2")
                        nc.scalar.activation(
                            scr2[:], dd[:], AF.Square, accum_out=vs[:]
                        )
                        vs2 = smpool.tile([128, 1], F32, tag="vs2")
                        nc.vector.tensor_scalar(
                            vs2[:], vs[:], 1.0 / F, EPS, op0=ALU.mult, op1=ALU.add
                        )
                        sd = smpool.tile([128, 1], F32, tag="sd")
                        nc.scalar.activation(sd[:], vs2[:], AF.Sqrt)
                        rstd = smpool.tile([128, 1], F32, tag="rstd")
                        nc.vector.reciprocal(rstd[:], sd[:])
                        xn = postpool.tile([128, F], F32, tag="xn")
                        nc.vector.tensor_scalar_mul(xn[:], dd[:], rstd[:])
                        xg = postpool.tile([128, F], F32, tag="xg")
                        nc.vector.tensor_tensor(xg[:], xn[:], gmb_s[:], op=ALU.mult)
                        xgb = postpool.tile([128, F], F32, tag="xgb")
                        nc.vector.tensor_tensor(xgb[:], xg[:], btb_s[:], op=ALU.add)
                        nc.scalar.activation(
                            x1_s[:, t * F : (t + 1) * F],
                            xgb[:],
                            AF.Lrelu,
                            alpha=SLOPE,
                        )
                    else:
                        q = postpool.tile([128, F], F32, tag="a0")
                        for hd in range(H):
                            nc.vector.tensor_scalar_mul(
                                q[:, hd * D : (hd + 1) * D],
                                ps[:, hd * D : (hd + 1) * D],
                                rec[:, hd : hd + 1],
                            )
                        p01 = postpool.tile([128, D], F32, tag="p01")
                        p23 = postpool.tile([128, D], F32, tag="p23")
                        nc.vector.tensor_tensor(
                            p01[:], q[:, 0:D], q[:, D : 2 * D], op=ALU.add
                        )
                        nc.vector.tensor_tensor(
                            p23[:], q[:, 2 * D : 3 * D], q[:, 3 * D : 4 * D], op=ALU.add
                        )
                        o = postpool.tile([128, D], F32, tag="o")
                        nc.vector.tensor_tensor(o[:], p01[:], p23[:], op=ALU.add)
                        nc.vector.tensor_scalar_mul(o[:], o[:], 0.25)
                        nc.sync.dma_start(
                            out=out_d[t * 128 : t * 128 + rows, :], in_=o[:rows, :]
                        )

            phases = os.environ.get("KPHASES", "b0,a0,b1,a1").split(",")
            if "b0" in phases:
                build_layer(0)
            if "a0" in phases:
                agg_layer(0)
            if "b1" in phases:
                build_layer(1)
            if "a1" in phases:
                agg_layer(1)
            if "a1" not in phases:
                # dummy output so the ExternalOutput is written
                zt = postpool.tile([128, D], F32, tag="o")
                nc.vector.memset(zt[:], 0.0)
                nc.sync.dma_start(out=out_d[0:128, :], in_=zt[:])

    nc.compile()
    return nc


_CACHE = {}


def kernel(**inputs):
    global LAST_RESULTS
    in_maps, NB1, NB2 = _host_prep(**inputs)
    key = (NB1, NB2, os.environ.get("KPHASES", "b0,a0,b1,a1"))
    if key not in _CACHE:
        _CACHE[key] = _build_program(NB1, NB2)
    nc = _CACHE[key]
    trace = bool(os.environ.get("BASS_TRACE"))
    res = run_bass_kernel_spmd(nc, in_maps, list(range(NCORE)), trace=trace)
    LAST_RESULTS = res
    out = np.concatenate([res.results[c]["out"] for c in range(NCORE)], axis=0)
    return out.astype(np.float32)


# revision 11
# speedup vs baseline: 1.0399x; 1.0399x over previous
"""GAT-style 2-layer GNN message passing on 8 Trainium2 NeuronCores.

Math note: for this reference, the segment-softmax ratio
  num/den = (sum_j h[j]*exp((s_l[i]+s_r[j])/2d)) / (sum_j exp((s_l[i]+s_r[j])/2d))
has the destination factor exp(s_l[i]/2d) cancel, so per layer we only need
  a[i] = (sum_{j in N(i)} w_j*h_j) / (sum_{j in N(i)} w_j),  w_j = exp(s_r[j]/2d).

Sharding: nodes split into 8 contiguous destination ranges (6250/core).
Each core builds table rows [g=w*h (256) | w (4) | pad] (bf16, 768B) for its
own nodes, an AllGather shares the full 50k-row table, then each core
aggregates its own destinations: per-edge dma_gather of source rows + one-hot
matmul (segment-sum into PSUM), then divide / layernorm / leaky-relu.
Edges are pre-sorted by destination on the host; indices are int16 so the
table is gathered via two base pointers (src < 32768 and src >= 32768).
"""

import os
import sys

import numpy as np
import ml_dtypes

sys.path.insert(0, "/opt/trn_rl_repo")

import concourse.bacc as bacc
import concourse.bass as bass
import concourse.mybir as mybir
import concourse.tile as tile
from concourse.bass_utils import run_bass_kernel_spmd

BF16 = mybir.dt.bfloat16
F32 = mybir.dt.float32
I16 = mybir.dt.int16

N, DIN, E = 50000, 128, 800000
H, D = 4, 64
F = H * D  # 256
NCORE = 8
NPC = N // NCORE  # 6250
NBLK = (NPC + 127) // 128  # 49 destination blocks per core
EPS = 1e-5
SLOPE = 0.01
ROWE = 384  # table row: 256 g + 4 w + 124 pad (bf16) = 768 bytes
SPLIT = 32768  # int16 index limit

LAST_RESULTS = None

AF = mybir.ActivationFunctionType
ALU = mybir.AluOpType


def _host_prep(x, edge_index, W0, b0, W1, b1, att0, att1, gamma, beta):
    """Build all per-core and shared input arrays."""
    bf16 = ml_dtypes.bfloat16
    dst = np.asarray(edge_index[0], dtype=np.int64)
    src = np.asarray(edge_index[1], dtype=np.int64)
    x = np.asarray(x, dtype=np.float32)

    # --- per-core edge grids ---
    per_core = []
    nb1_max, nb2_max = 0, 0
    for c in range(NCORE):
        lo_node, hi_node = c * NPC, (c + 1) * NPC
        m = (dst >= lo_node) & (dst < hi_node)
        ld = (dst[m] - lo_node).astype(np.int64)
        s = src[m]
        blocks = []
        for b in range(NBLK):
            bm = (ld >> 7) == b
            sb = s[bm]
            lb = (ld[bm] & 127).astype(np.int64)
            lo_m = sb < SPLIT
            runs = (
                (sb[lo_m], lb[lo_m]),
                (sb[~lo_m] - SPLIT, lb[~lo_m]),
            )
            blocks.append(runs)
            nb1_max = max(nb1_max, -(-len(runs[0][0]) // 128))
            nb2_max = max(nb2_max, -(-len(runs[1][0]) // 128))
        per_core.append(blocks)
    NB1, NB2 = max(nb1_max, 1), max(nb2_max, 1)
    NBT = NB1 + NB2

    eidx_all, dloc_all = [], []
    for c in range(NCORE):
        eidx = np.zeros((128, NBLK * NBT * 8), dtype=np.int16)
        dloc = np.full((128, NBLK * NBT), 255.0, dtype=bf16)
        for b in range(NBLK):
            for r, nb in ((0, NB1), (1, NB2)):
                sidx, lb = per_core[c][b][r]
                n = len(sidx)
                pad = nb * 128 - n
                si = np.concatenate([sidx, np.zeros(pad, np.int64)]).astype(np.int16)
                dl = np.concatenate([lb, np.full(pad, 255, np.int64)]).astype(
                    np.float32
                )
                c0 = (b * NBT + (0 if r == 0 else NB1)) * 8
                eidx[:, c0 : c0 + nb * 8] = np.tile(
                    si.reshape(nb * 8, 16).T, (8, 1)
                )
                cb = b * NBT + (0 if r == 0 else NB1)
                dloc[:, cb : cb + nb] = dl.reshape(nb, 128).T.astype(bf16)
        eidx_all.append(eidx)
        dloc_all.append(dloc)

    # --- xT slices (bf16, padded to 49*128 columns) ---
    xts = []
    for c in range(NCORE):
        xt = np.zeros((DIN, NBLK * 128), dtype=bf16)
        xt[:, :NPC] = x[c * NPC : (c + 1) * NPC].T.astype(bf16)
        xts.append(xt)

    shared = {
        "w0t": np.ascontiguousarray(np.asarray(W0, np.float32).T).astype(bf16),
        "w1t": np.ascontiguousarray(np.asarray(W1, np.float32).T).astype(bf16),
        "b0b": np.tile(np.asarray(b0, np.float32)[None, :], (128, 1)),
        "b1b": np.tile(np.asarray(b1, np.float32)[None, :], (128, 1)),
        "ar0": np.tile(
            np.asarray(att0, np.float32)[0, :, D:].reshape(-1)[None, :], (128, 1)
        ),
        "ar1": np.tile(
            np.asarray(att1, np.float32)[0, :, D:].reshape(-1)[None, :], (128, 1)
        ),
        "gmb": np.tile(np.asarray(gamma, np.float32)[None, :], (128, 1)),
        "btb": np.tile(np.asarray(beta, np.float32)[None, :], (128, 1)),
        "iot": np.tile(
            np.tile(np.arange(128, dtype=np.float32), max(NB1, NB2))[None, :],
            (128, 1),
        ).astype(bf16),
        "idn": np.eye(128, dtype=np.float32).astype(bf16),
    }
    in_maps = []
    for c in range(NCORE):
        m = dict(shared)
        m["xt"] = xts[c]
        m["eidx"] = eidx_all[c]
        m["dloc"] = dloc_all[c]
        in_maps.append(m)
    return in_maps, NB1, NB2


def _build_program(NB1, NB2):
    NBT = NB1 + NB2
    nc = bacc.Bacc("TRN2", target_bir_lowering=False, debug=False, num_devices=NCORE)

    # I/O
    xt_d = nc.dram_tensor("xt", [DIN, NBLK * 128], BF16, kind="ExternalInput")
    eidx_d = nc.dram_tensor("eidx", [128, NBLK * NBT * 8], I16, kind="ExternalInput")
    dloc_d = nc.dram_tensor("dloc", [128, NBLK * NBT], BF16, kind="ExternalInput")
    w0t_d = nc.dram_tensor("w0t", [DIN, F], BF16, kind="ExternalInput")
    w1t_d = nc.dram_tensor("w1t", [F, F], BF16, kind="ExternalInput")
    b0b_d = nc.dram_tensor("b0b", [128, F], F32, kind="ExternalInput")
    b1b_d = nc.dram_tensor("b1b", [128, F], F32, kind="ExternalInput")
    ar0_d = nc.dram_tensor("ar0", [128, F], F32, kind="ExternalInput")
    ar1_d = nc.dram_tensor("ar1", [128, F], F32, kind="ExternalInput")
    gmb_d = nc.dram_tensor("gmb", [128, F], F32, kind="ExternalInput")
    btb_d = nc.dram_tensor("btb", [128, F], F32, kind="ExternalInput")
    iot_d = nc.dram_tensor("iot", [128, max(NB1, NB2) * 128], BF16, kind="ExternalInput")
    idn_d = nc.dram_tensor("idn", [128, 128], BF16, kind="ExternalInput")
    out_d = nc.dram_tensor("out", [NPC, D], F32, kind="ExternalOutput")

    # Internal DRAM
    tbl_own = [nc.dram_tensor(f"tbl_own{l}", [NPC, ROWE], BF16) for l in range(2)]
    tbl_full = [
        nc.dram_tensor(f"tbl_full{l}", [N, ROWE], BF16, addr_space="Shared")
        for l in range(2)
    ]

    groups = [list(range(NCORE))]

    with tile.TileContext(nc) as tc:
        with (
            tc.tile_pool(name="const", bufs=1) as cpool,
            tc.tile_pool(name="stat", bufs=3) as spool,
            tc.tile_pool(name="hbuf", bufs=3) as hpool,
            tc.tile_pool(name="small", bufs=4) as smpool,
            tc.tile_pool(name="tblt", bufs=3) as tbpool,
            tc.tile_pool(name="glo", bufs=2) as glopool,
            tc.tile_pool(name="ghi", bufs=2) as ghipool,
            tc.tile_pool(name="oh", bufs=4) as ohpool,
            tc.tile_pool(name="post", bufs=3) as postpool,
            tc.tile_pool(name="gemm", bufs=2, space="PSUM") as gpsum,
            tc.tile_pool(name="agg", bufs=2, space="PSUM") as apsum,
            tc.tile_pool(name="tp", bufs=2, space="PSUM") as tpsum,
        ):
            # ---- load constants ----
            def cload(dram, shape, dtype):
                t = cpool.tile(shape, dtype, tag=dram.name)
                nc.sync.dma_start(out=t[:], in_=dram[:, :])
                return t

            xt_s = cload(xt_d, [DIN, NBLK * 128], BF16)
            eidx_s = cload(eidx_d, [128, NBLK * NBT * 8], I16)
            dloc_s = cload(dloc_d, [128, NBLK * NBT], BF16)
            w0t_s = cload(w0t_d, [DIN, F], BF16)
            w1t_s = []
            for cch in range(2):
                t = cpool.tile([128, F], BF16, tag=f"w1t{cch}")
                nc.sync.dma_start(
                    out=t[:], in_=w1t_d[cch * 128 : (cch + 1) * 128, :]
                )
                w1t_s.append(t)
            b0b_s = cload(b0b_d, [128, F], F32)
            b1b_s = cload(b1b_d, [128, F], F32)
            ar0_s = cload(ar0_d, [128, F], F32)
            ar1_s = cload(ar1_d, [128, F], F32)
            gmb_s = cload(gmb_d, [128, F], F32)
            btb_s = cload(btb_d, [128, F], F32)
            iot_s = cload(iot_d, [128, max(NB1, NB2) * 128], BF16)
            idn_s = cload(idn_d, [128, 128], BF16)
            x1_s = cpool.tile([128, NBLK * F], BF16, tag="x1")
            gt_bufs = {}
            for r, nb in ((0, NB1), (1, NB2)):
                for i in range(2):
                    t = cpool.tile([128, nb, ROWE], BF16, tag=f"gt{r}_{i}")
                    nc.vector.memset(t[:], 0.0)
                    gt_bufs[(r, i)] = t

            def build_layer(l):
                b_s = b0b_s if l == 0 else b1b_s
                ar_s = ar0_s if l == 0 else ar1_s
                for t in range(NBLK):
                    rows = min(128, NPC - t * 128)
                    ps = gpsum.tile([128, F], F32, tag="gemm")
                    if l == 0:
                        nc.tensor.matmul(
                            ps[:],
                            lhsT=xt_s[:, t * 128 : (t + 1) * 128],
                            rhs=w0t_s[:],
                            start=True,
                            stop=True,
                        )
                    else:
                        for cch in range(2):
                            pt = tpsum.tile([128, 128], BF16, tag="tp")
                            nc.tensor.transpose(
                                pt[:],
                                x1_s[:, t * F + cch * 128 : t * F + (cch + 1) * 128],
                                idn_s[:],
                            )
                            st = spool.tile([128, 128], BF16, tag="stat")
                            nc.vector.tensor_copy(st[:], pt[:])
                            nc.tensor.matmul(
                                ps[:],
                                lhsT=st[:],
                                rhs=w1t_s[cch][:],
                                start=(cch == 0),
                                stop=(cch == 1),
                            )
                    h = hpool.tile([128, F], F32, tag="h")
                    nc.vector.tensor_tensor(h[:], ps[:], b_s[:], op=ALU.add)
                    u = hpool.tile([128, F], F32, tag="u")
                    nc.vector.tensor_tensor(u[:], h[:], ar_s[:], op=ALU.mult)
                    sr = smpool.tile([128, H], F32, tag="sr")
                    scr = hpool.tile([128, F], F32, tag="scr")
                    for hd in range(H):
                        nc.scalar.activation(
                            scr[:, hd * D : (hd + 1) * D],
                            u[:, hd * D : (hd + 1) * D],
                            AF.Lrelu,
                            alpha=SLOPE,
                            accum_out=sr[:, hd : hd + 1],
                        )
                    wv = smpool.tile([128, H], F32, tag="wv")
                    nc.scalar.activation(wv[:], sr[:], AF.Exp, scale=1.0 / (2 * D))
                    tb = tbpool.tile([128, F + H], BF16, tag="tb")
                    for hd in range(H):
                        nc.vector.tensor_scalar_mul(
                            tb[:, hd * D : (hd + 1) * D],
                            h[:, hd * D : (hd + 1) * D],
                            wv[:, hd : hd + 1],
                        )
                    nc.vector.tensor_copy(tb[:, F : F + H], wv[:])
                    nc.sync.dma_start(
                        out=tbl_own[l][t * 128 : t * 128 + rows, 0 : F + H],
                        in_=tb[:rows, :],
                    )
                nc.gpsimd.collective_compute(
                    "AllGather",
                    ALU.bypass,
                    replica_groups=groups,
                    ins=[tbl_own[l][:, :]],
                    outs=[tbl_full[l][:, :]],
                )

            def agg_layer(l):
                for t in range(NBLK):
                    rows = min(128, NPC - t * 128)
                    ps = apsum.tile([128, F + H], F32, tag="agg")
                    nmm = 0
                    for r, nb in ((0, NB1), (1, NB2)):
                        gt = gt_bufs[(r, t % 2)]
                        base = (
                            tbl_full[l][0:SPLIT, :]
                            if r == 0
                            else tbl_full[l][SPLIT:N, :]
                        )
                        c0 = (t * NBT + (0 if r == 0 else NB1)) * 8
                        nc.gpsimd.dma_gather(
                            gt[:],
                            base,
                            eidx_s[:, c0 : c0 + nb * 8],
                            nb * 128,
                            nb * 128,
                            ROWE,
                            single_packet=(nb * 128 <= 1024),
                        )
                        cb = t * NBT + (0 if r == 0 else NB1)
                        oh = ohpool.tile([128, nb * 128], BF16, tag=f"oh{r}")
                        nc.vector.tensor_tensor(
                            oh[:].rearrange("p (a b) -> p a b", b=128),
                            iot_s[:, 0 : nb * 128].rearrange(
                                "p (a b) -> p a b", b=128
                            ),
                            dloc_s[:, cb : cb + nb].to_broadcast((128, nb, 128)),
                            op=ALU.is_equal,
                        )
                        for b in range(nb):
                            nc.tensor.matmul(
                                ps[:],
                                lhsT=oh[:, b * 128 : (b + 1) * 128],
                                rhs=gt[:, b, 0 : F + H],
                                start=(nmm == 0),
                                stop=(nmm == NBT - 1),
                            )
                            nmm += 1
                    rec = smpool.tile([128, H], F32, tag="rec")
                    nc.vector.reciprocal(rec[:], ps[:, F : F + H])
                    if l == 0:
                        a0 = postpool.tile([128, F], F32, tag="a0")
                        for hd in range(H):
                            nc.vector.tensor_scalar_mul(
                                a0[:, hd * D : (hd + 1) * D],
                                ps[:, hd * D : (hd + 1) * D],
                                rec[:, hd : hd + 1],
                            )
                        mu = smpool.tile([128, 1], F32, tag="mu")
                        nc.vector.tensor_reduce(
                            mu[:], a0[:], axis=mybir.AxisListType.X, op=ALU.add
                        )
                        nc.vector.tensor_scalar_mul(mu[:], mu[:], 1.0 / F)
                        dd = postpool.tile([128, F], F32, tag="dd")
                        nc.vector.tensor_scalar_sub(dd[:], a0[:], mu[:])
                        vs = smpool.tile([128, 1], F32, tag="vs")
                        scr2 = postpool.tile([128, F], F32, tag="scr2")
                        nc.scalar.activation(
                            scr2[:], dd[:], AF.Square, accum_out=vs[:]
                        )
                        vs2 = smpool.tile([128, 1], F32, tag="vs2")
                        nc.vector.tensor_scalar(
                            vs2[:], vs[:], 1.0 / F, EPS, op0=ALU.mult, op1=ALU.add
                        )
                        sd = smpool.tile([128, 1], F32, tag="sd")
                        nc.scalar.activation(sd[:], vs2[:], AF.Sqrt)
                        rstd = smpool.tile([128, 1], F32, tag="rstd")
                        nc.vector.reciprocal(rstd[:], sd[:])
                        xn = postpool.tile([128, F], F32, tag="xn")
                        nc.vector.tensor_scalar_mul(xn[:], dd[:], rstd[:])
                        xg = postpool.tile([128, F], F32, tag="xg")
                        nc.vector.tensor_tensor(xg[:], xn[:], gmb_s[:], op=ALU.mult)
                        xgb = postpool.tile([128, F], F32, tag="xgb")
                        nc.vector.tensor_tensor(xgb[:], xg[:], btb_s[:], op=ALU.add)
                        nc.scalar.activation(
                            x1_s[:, t * F : (t + 1) * F],
                            xgb[:],
                            AF.Lrelu,
                            alpha=SLOPE,
                        )
                    else:
                        q = postpool.tile([128, F], F32, tag="a0")
                        for hd in range(H):
                            nc.vector.tensor_scalar_mul(
                                q[:, hd * D : (hd + 1) * D],
                                ps[:, hd * D : (hd + 1) * D],
                                rec[:, hd : hd + 1],
                            )
                        p01 = postpool.tile([128, D], F32, tag="p01")
                        p23 = postpool.tile([128, D], F32, tag="p23")
                        nc.vector.tensor_tensor(
                            p01[:], q[:, 0:D], q[:, D : 2 * D], op=ALU.add
                        )
                        nc.vector.tensor_tensor(
                            p23[:], q[:, 2 * D : 3 * D], q[:, 3 * D : 4 * D], op=ALU.add
                        )
                        o = postpool.tile([128, D], F32, tag="o")
                        nc.vector.tensor_tensor(o[:], p01[:], p23[:], op=ALU.add)
                        nc.vector.tensor_scalar_mul(o[:], o[:], 0.25)
                        nc.sync.dma_start(
                            out=out_d[t * 128 : t * 128 + rows, :], in_=o[:rows, :]
                        )

            phases = os.environ.get("KPHASES", "b0,a0,b1,a1").split(",")
            if "b0" in phases:
                build_layer(0)
            if "a0" in phases:
                agg_layer(0)
            if "b1" in phases:
                build_layer(1)
            if "a1" in phases:
                agg_layer(1)
            if "a1" not in phases:
                # dummy output so the ExternalOutput is written
                zt = postpool.tile([128, D], F32, tag="o")
                nc.vector.memset(zt[:], 0.0)
                nc.sync.dma_start(out=out_d[0:128, :], in_=zt[:])

    nc.compile()
    return nc


_CACHE = {}


def kernel(**inputs):
    global LAST_RESULTS
    in_maps, NB1, NB2 = _host_prep(**inputs)
    key = (NB1, NB2, os.environ.get("KPHASES", "b0,a0,b1,a1"))
    if key not in _CACHE:
        _CACHE[key] = _build_program(NB1, NB2)
    nc = _CACHE[key]
    trace = bool(os.environ.get("BASS_TRACE"))
    res = run_bass_kernel_spmd(nc, in_maps, list(range(NCORE)), trace=trace)
    LAST_RESULTS = res
    out = np.concatenate([res.results[c]["out"] for c in range(NCORE)], axis=0)
    return out.astype(np.float32)
